# revision 2
# baseline (speedup 1.0000x reference)
"""Trainium2 Bass kernel for nn_Attention_39865886442202 (sparse periodic local attention).

Strategy:
  - Data-parallel over batch B=8 across 8 NeuronCores (one batch element per core).
  - Mask is periodic with period w=128: key j visible to query i iff (j mod 128) is in an
    11-wide (clamped) band around (i mod 128). Each query attends to <=208 of 2048 keys.
  - Scores are computed in "ST" layout (keys on partitions, query-rows on free dim) so the
    AV matmul needs no transposes and the softmax denominator comes from an appended
    ones-column in V.
  - Keys/queries are regrouped by residue (c-major / p-major) via strided access patterns.
  - The additive mask is applied with a rank-8 accumulating matmul (maskT[8,keys] x
    onehot[8,r]) into the same PSUM bank as the QK product.
  - exp() without max-subtraction (scores are provably small for this problem size).
"""

import math

import ml_dtypes
import numpy as np

import concourse.bass as bass
import concourse.mybir as mybir
import concourse.tile as tile
from concourse import bacc, bass_utils

DIM = 256
NUM_HEADS = 8
HEAD_DIM = 32
SCALE = HEAD_DIM ** (-0.5)
W_BUF = 160
WK = 11
B = 8
N = 2048
W = 128           # w
T = N // W        # 16 key/query blocks
PT = 8            # residues per tile
NTILES = W // PT  # 16 tiles
NEG = -30000.0

_CACHE = {}
LAST_EXEC_NS = None


def _build_m1(w):
    """[w, w] 0/-inf base mask, copied from the reference construction."""
    Wb = W_BUF
    mask = np.ones((Wb, Wb), dtype=np.float32)
    for i in range(Wb):
        b = i - WK // 2 if i - WK // 2 > 0 else 0
        if b > Wb - WK:
            b = Wb - WK
        mask[i, b:b + WK] = 0.0
    mask[mask >= 1] = -np.inf
    off = (Wb - w) // 2
    m1 = mask[off:Wb - off, off:Wb - off].copy()
    e = WK // 2 + 1
    m1[:e] = mask[:e, :w]
    m1[-e:] = mask[-e:, -w:]
    return m1  # [128, 128]


def _host_consts():
    m1 = _build_m1(W)  # m1[p, c] == 0 if visible else -inf
    # maskT[k, j, ci, key]: tile k (p0 = 8k), row j (p = 8k + j), chunk slot ci
    # (global chunk cg = k - 1 + ci, keys c = 8*cg + key//16, s = key%16).
    maskT = np.full((NTILES, PT, 3, 128), NEG, dtype=np.float32)
    for k in range(NTILES):
        for ci in range(3):
            cg = k - 1 + ci
            if cg < 0 or cg >= 16:
                continue
            for j in range(PT):
                p = 8 * k + j
                for cl in range(8):
                    c = 8 * cg + cl
                    val = 0.0 if m1[p, c] == 0.0 else NEG
                    maskT[k, j, ci, cl * 16:(cl + 1) * 16] = val
    onehot = np.zeros((PT, 128), dtype=np.float32)
    for j in range(PT):
        onehot[j, j * 16:(j + 1) * 16] = 1.0
    ident = np.eye(128, dtype=np.float32)
    bf = ml_dtypes.bfloat16
    return maskT.reshape(PT, -1, order="F"), onehot.astype(bf), ident.astype(bf)


def _maskt_flat(maskT4):
    # layout [8, NTILES*3*128]: row j, col (k*3 + ci)*128 + key
    out = np.zeros((PT, NTILES * 3 * 128), dtype=np.float32)
    return out


def _build_consts():
    m1 = _build_m1(W)
    maskT = np.full((PT, NTILES * 3 * 128), NEG, dtype=np.float32)
    for k in range(NTILES):
        for ci in range(3):
            cg = k - 1 + ci
            if cg < 0 or cg >= 16:
                continue
            for j in range(PT):
                p = 8 * k + j
                for cl in range(8):
                    c = 8 * cg + cl
                    if m1[p, c] == 0.0:
                        base = (k * 3 + ci) * 128 + cl * 16
                        maskT[j, base:base + 16] = 0.0
    onehot = np.zeros((PT, 128), dtype=np.float32)
    for j in range(PT):
        onehot[j, j * 16:(j + 1) * 16] = 1.0
    ident = np.eye(128, dtype=np.float32)
    bf = ml_dtypes.bfloat16
    return maskT.astype(bf), onehot.astype(bf), ident.astype(bf)


def _chunks_for_tile(k):
    return [c for c in (k - 1, k, k + 1) if 0 <= c < 16]


def _build_program():
    # r-window per (tile, chunk): rows that can contain valid scores
    m1 = _build_m1(W)
    rwin = {}
    for k in range(NTILES):
        for cg in _chunks_for_tile(k):
            prs = [j for j in range(PT)
                   if any(m1[8 * k + j, c] == 0.0 for c in range(8 * cg, 8 * cg + 8))]
            rwin[(k, cg)] = (16 * min(prs), 16 * (max(prs) + 1))
    nc = bacc.Bacc(None, target_bir_lowering=False)
    f32 = mybir.dt.float32
    bf16 = mybir.dt.bfloat16

    x_in = nc.declare_dram_parameter("x", [N, DIM], f32, isOutput=False)
    wqkv_in = nc.declare_dram_parameter("wqkv", [DIM, 3 * DIM], f32, isOutput=False)
    wproj_in = nc.declare_dram_parameter("wproj", [DIM, DIM], f32, isOutput=False)
    bproj_in = nc.declare_dram_parameter("bproj", [DIM], f32, isOutput=False)
    maskt_in = nc.declare_dram_parameter("maskt", [PT, NTILES * 3 * 128], bf16,
                                         isOutput=False)
    onehot_in = nc.declare_dram_parameter("onehot", [PT, 128], bf16, isOutput=False)
    ident_in = nc.declare_dram_parameter("ident", [128, 128], bf16, isOutput=False)
    out_ext = nc.declare_dram_parameter("out", [N, DIM], f32, isOutput=True)

    with tile.TileContext(nc) as tc:
        with (
            tc.tile_pool(name="singles", bufs=1) as singles,
            tc.tile_pool(name="ptilp", bufs=3) as ptilp,
            tc.tile_pool(name="smallp", bufs=4) as smallp,
            tc.tile_pool(name="psA", bufs=2, space="PSUM") as psA,
            tc.tile_pool(name="psS", bufs=3, space="PSUM") as psS,
            tc.tile_pool(name="psO", bufs=2, space="PSUM") as psO,
        ):
            # ---- constants / weights to SBUF (bf16 casts via SWDGE) ----
            # mask/onehot replicated at partition bases 0/32/64/96 so the mask
            # matmul shares the QK matmul's PE row group (PSUM bank safety).
            maskt_sb = singles.tile([128, NTILES * 3 * 128], bf16)
            onehot_sb = singles.tile([128, 128], bf16)
            for g in range(4):
                nc.sync.dma_start(out=maskt_sb[32 * g:32 * g + PT, :], in_=maskt_in[:, :])
                nc.sync.dma_start(out=onehot_sb[32 * g:32 * g + PT, :], in_=onehot_in[:, :])
            ident_sb = singles.tile([128, 128], bf16)
            nc.sync.dma_start(out=ident_sb, in_=ident_in[:, :])

            wqkv_sb = []
            for dchunk in range(2):
                t_ = singles.tile([128, 3 * DIM], bf16, tag=f"wqkv{dchunk}")
                nc.gpsimd.dma_start(out=t_, in_=wqkv_in[128 * dchunk:128 * (dchunk + 1), :])
                wqkv_sb.append(t_)
            wproj_sb = []
            for dchunk in range(2):
                t_ = singles.tile([128, DIM], bf16, tag=f"wproj{dchunk}")
                nc.gpsimd.dma_start(out=t_, in_=wproj_in[128 * dchunk:128 * (dchunk + 1), :])
                wproj_sb.append(t_)
            bproj_rep = singles.tile([128, DIM], f32)
            bp = bproj_in[:]
            bproj_bcast = bass.AP(tensor=bp.tensor, offset=bp.offset,
                                  ap=[[0, 128], [1, DIM]])
            nc.gpsimd.dma_start(out=bproj_rep, in_=bproj_bcast)

            # ---- x -> bf16 SBUF (row tiles side by side), then PE-transpose to xT ----
            xbf = singles.tile([128, T * DIM], bf16)
            xin = x_in.rearrange("(t p) d -> p t d", p=128)
            nc.gpsimd.dma_start(out=xbf.rearrange("p (t d) -> p t d", t=T), in_=xin)
            xT = []
            for dchunk in range(2):
                t_ = singles.tile([128, N], bf16, tag=f"xT{dchunk}")
                xT.append(t_)
            for t in range(T):
                for dchunk in range(2):
                    tp = psA.tile([128, 128], bf16, tag="ps")
                    nc.tensor.transpose(
                        tp, xbf[:, 256 * t + 128 * dchunk:256 * t + 128 * (dchunk + 1)],
                        ident_sb)
                    nc.vector.tensor_copy(xT[dchunk][:, 128 * t:128 * (t + 1)], tp)

            # ---- qkvT projection into residue-major layouts ----
            # qkT[oc][128, N] with column = p*16 + t  (token n = t*128 + p)
            qkT = [singles.tile([128, N], bf16, name=f"qkT{i}", tag=f"qkT{i}") for i in range(4)]
            NF = 512
            for oc in range(4):
                for nf in range(N // NF):
                    ps = psA.tile([128, NF], f32)
                    for dc in range(2):
                        nc.tensor.matmul(
                            ps,
                            lhsT=wqkv_sb[dc][:, 128 * oc:128 * (oc + 1)],
                            rhs=xT[dc][:, NF * nf:NF * (nf + 1)],
                            start=(dc == 0), stop=(dc == 1),
                        )
                    src = bass.AP(tensor=ps.tensor, offset=ps.offset,
                                  ap=[list(ps.ap[0]), [128, 4], [1, 128]])
                    dtile = qkT[oc][:, :]
                    dst = bass.AP(tensor=dtile.tensor, offset=dtile.offset + 4 * nf,
                                  ap=[list(dtile.ap[0]), [1, 4], [16, 128]])
                    nc.vector.tensor_copy(dst, src)

            # xTg: x^T with key-major columns (col = c*16 + s, token n = s*128 + c)
            xTg = []
            for dc in range(2):
                t_ = singles.tile([128, N], bf16, tag=f"xTg{dc}")
                st = xT[dc][:, :]
                src = bass.AP(tensor=st.tensor, offset=st.offset,
                              ap=[list(st.ap[0]), [128, 16], [1, 128]])
                dt_ = t_[:, :]
                dst = bass.AP(tensor=dt_.tensor, offset=dt_.offset,
                              ap=[list(dt_.ap[0]), [1, 16], [16, 128]])
                nc.vector.tensor_copy(dst, src)
                xTg.append(t_)

            # ---- V projection in key-major chunks: vsb[128, 16*264] ----
            # chunk m: keys (c in [8m,8m+8), s in [0,16)), cols 264*m + 33*h + d, ones at d=32
            vsb = singles.tile([128, 16 * 264], bf16)
            vsb4 = vsb.rearrange("p (m h e) -> p m h e", m=16, e=33)
            nc.vector.memset(vsb4[:, :, :, 32:33], 1.0)
            for m in range(16):
                ps = psA.tile([128, DIM], f32)
                for dc in range(2):
                    nc.tensor.matmul(
                        ps,
                        lhsT=xTg[dc][:, 128 * m:128 * (m + 1)],
                        rhs=wqkv_sb[dc][:, 2 * DIM:3 * DIM],
                        start=(dc == 0), stop=(dc == 1),
                    )
                nc.vector.tensor_copy(
                    vsb4[:, m, :, 0:32],
                    ps.rearrange("p (h e) -> p h e", h=NUM_HEADS),
                )

            # ---- attention ----
            attnout = [singles.tile([128, DIM], bf16, name=f"ao{i}", tag=f"ao{i}") for i in range(NTILES)]
            for k in range(NTILES):
                cks = _chunks_for_tile(k)
                ncks = len(cks)
                av_ps = psO.tile([128, NUM_HEADS * 33], f32)
                for h in range(NUM_HEADS):
                    qtile = qkT[h // 4]
                    ktile = qkT[2 + h // 4]
                    base = (32 * h) % 128
                    s_ps = psS.tile([128, 384], f32)
                    for j, cg in enumerate(cks):
                        nc.tensor.matmul(
                            s_ps[:, 128 * j:128 * (j + 1)],
                            lhsT=ktile[base:base + 32, 128 * cg:128 * (cg + 1)],
                            rhs=qtile[base:base + 32, 128 * k:128 * (k + 1)],
                            start=True, stop=False,
                            tile_position=(base, 0),
                        )
                        ci = cg - (k - 1)
                        moff = (k * 3 + ci) * 128
                        nc.tensor.matmul(
                            s_ps[:, 128 * j:128 * (j + 1)],
                            lhsT=maskt_sb[base:base + PT, moff:moff + 128],
                            rhs=onehot_sb[base:base + PT, :],
                            start=False, stop=True,
                            tile_position=(base, 0),
                        )
                    ptil = ptilp.tile([128, 384], bf16)
                    nc.scalar.activation(
                        ptil[:, :128 * ncks], s_ps[:, :128 * ncks],
                        mybir.ActivationFunctionType.Exp,
                    )
                    for j, cg in enumerate(cks):
                        nc.tensor.matmul(
                            av_ps[:, 33 * h:33 * (h + 1)],
                            lhsT=ptil[:, 128 * j:128 * (j + 1)],
                            rhs=vsb[:, 264 * cg + 33 * h:264 * cg + 33 * (h + 1)],
                            start=(j == 0), stop=(j == ncks - 1),
                        )
                # normalize: attnout[k][:, 32h+d] = av[:, 33h+d] * (1/av[:, 33h+32])
                av3 = av_ps.rearrange("p (h e) -> p h e", e=33)
                zrec = smallp.tile([128, NUM_HEADS], f32, tag="zrec")
                nc.vector.reciprocal(zrec, av3[:, :, 32])
                zr = zrec[:, :]
                zb = bass.AP(tensor=zr.tensor, offset=zr.offset,
                             ap=[list(zr.ap[0]), [1, NUM_HEADS], [0, 32]])
                nc.vector.tensor_mul(
                    attnout[k].rearrange("p (h e) -> p h e", e=32),
                    av3[:, :, 0:32],
                    zb,
                )

            # ---- transpose attnout -> attnoutT [256, 2048] ----
            # aoT[fc][128 f, 2048 n] in TOKEN order: col n = 128*t + 8*k + p_rel
            aoT = [singles.tile([128, N], bf16, name=f"aoT{i}", tag=f"aoT{i}") for i in range(2)]
            for k in range(NTILES):
                for fc in range(2):
                    tp = psA.tile([128, 128], bf16, tag="ps")
                    nc.tensor.transpose(
                        tp, attnout[k][:, 128 * fc:128 * (fc + 1)], ident_sb)
                    st = tp[:, :]
                    src = bass.AP(tensor=st.tensor, offset=st.offset,
                                  ap=[list(st.ap[0]), [16, 8], [1, 16]])
                    dt_ = aoT[fc][:, :]
                    dst = bass.AP(tensor=dt_.tensor, offset=dt_.offset + 8 * k,
                                  ap=[list(dt_.ap[0]), [1, 8], [128, 16]])
                    nc.vector.tensor_copy(dst, src)

            # ---- final projection + bias; rows n = 128*t' + 8k + p_rel ----
            for tp_ in range(T):
                ps = psA.tile([128, DIM], f32, tag="ps")
                for fc in range(2):
                    nc.tensor.matmul(
                        ps,
                        lhsT=aoT[fc][:, 128 * tp_:128 * (tp_ + 1)],
                        rhs=wproj_sb[fc][:, :],
                        start=(fc == 0), stop=(fc == 1),
                    )
                osb = smallp.tile([128, DIM], f32, tag="osb")
                nc.vector.tensor_add(osb, ps, bproj_rep)
                # row r = k*8... out partition r maps to n = 128*tp_ + (r//8)*8?? see below
                nc.sync.dma_start(
                    out=out_ext.rearrange("(t n) d -> t n d", t=T)[tp_, :, :],
                    in_=osb,
                )
    nc.finalize()
    return nc


def kernel(x, w, Wqkv, Wproj, bproj, **kw):
    global LAST_EXEC_NS
    import os
    assert int(w) == W
    x = np.asarray(x, dtype=np.float32)
    Wqkv = np.asarray(Wqkv, dtype=np.float32).copy()
    Wproj = np.asarray(Wproj, dtype=np.float32)
    bproj = np.asarray(bproj, dtype=np.float32)
    Wqkv[:, :DIM] = Wqkv[:, :DIM] * SCALE  # fold attention scale into Wq

    if "prog" not in _CACHE:
        _CACHE["prog"] = _build_program()
        _CACHE["consts"] = _build_consts()
    nc = _CACHE["prog"]
    maskT, onehot, ident = _CACHE["consts"]

    core_ids = list(range(B))
    in_maps = []
    for b in range(B):
        in_maps.append({
            "x": np.ascontiguousarray(x[b]),
            "wqkv": Wqkv,
            "wproj": Wproj,
            "bproj": bproj,
            "maskt": maskT,
            "onehot": onehot,
            "ident": ident,
        })
    res = bass_utils.run_bass_kernel_spmd(nc, in_maps, core_ids)
    globals()["LAST_RES"] = res
    LAST_EXEC_NS = res.exec_time_ns
    out = np.stack([res.results[b]["out"] for b in range(B)], axis=0)
    return out.astype(np.float32)

